# revision 55
# baseline (speedup 1.0000x reference)
"""Trainium2 Bass kernel for nn_Attn_30734785970994.

Dense transformer attention block with QK-norm (L2 + learned per-head scale),
cross/label tokens appended to K/V, NeoX rotary embedding, softmax attention,
and output projection.

Sharding (8 cores): 2-way data parallel over batch x 4-way tensor parallel
over heads (4 heads per core).  w_qkv / w_cross_qkv are split along their
output dim, w_out along its input dim (row-parallel); the per-core partial
outputs are summed on the host (the "all-reduce") during the gather step.

Per-core pipeline (matmuls in fp32r where operands are fp32; attention inner
loops in bf16; PSUM accumulates in fp32):
  P0: cross k/v projection (natural layout), QK-norm, rope, transpose
  P1: self q/k/v projection with fully-resident weights.  q/k are computed in
      natural (token-partition) layout so the L2 norm is a free-dim reduce and
      rope stays within a partition, then PE-transposed into (dh, token)
      layout.  kT/qT spill to a DRAM scratch (SBUF cannot hold them plus the
      resident weights); v stays resident in SBUF.
  P2: reload kT/qT; scores S^T = k^T.T @ q^T per 128-key block, exp on ScalarE
      (no max subtraction: scores are bounded by the QK norm, |s| < 0.1),
      softmax denominator via ones-stationary matmul (broadcast across
      partitions), o^T = v.T @ p^T, normalize by the reciprocal denominator,
      row-parallel output projection.
"""

import math
from contextlib import ExitStack

import ml_dtypes
import numpy as np

import concourse.bacc as bacc
import concourse.mybir as mybir
from concourse.alu_op_type import AluOpType
from concourse.bass_utils import run_bass_kernel_spmd
from concourse.masks import make_identity
from concourse.tile import TileContext

B, N, NCR, D, H = 2, 2048, 128, 2048, 16
DH = D // H            # 128
HG = 4                 # heads per core
NK = N + NCR           # 2176 keys
KB = NK // 128         # 17 key blocks
NCHUNK = D // 128      # 16 contraction chunks
ST = N // 512          # 4 seq tiles
F32 = mybir.dt.float32
F32R = mybir.dt.float32r
BF16 = mybir.dt.bfloat16
EXP_SCALE = DH ** -0.5
AF = mybir.ActivationFunctionType


def _build(reps=1):
    nc = bacc.Bacc(None, target_bir_lowering=False, debug=False)

    xT = nc.dram_tensor("xT", [D, N], F32R, kind="ExternalInput").ap()
    cT = nc.dram_tensor("cT", [D, NCR], BF16, kind="ExternalInput").ap()
    xTb = nc.dram_tensor("xTb", [D, N], BF16, kind="ExternalInput").ap()
    wqkT = nc.dram_tensor("wqkT", [D, 2 * HG * DH], BF16, kind="ExternalInput").ap()
    wvT = nc.dram_tensor("wvT", [D, HG * DH], F32R, kind="ExternalInput").ap()
    wckvT = nc.dram_tensor("wckvT", [D, 2 * HG * DH], BF16, kind="ExternalInput").ap()
    woutT = nc.dram_tensor("woutT", [HG * DH, D], F32R, kind="ExternalInput").ap()
    cosN = nc.dram_tensor("cosN", [NK, DH], BF16, kind="ExternalInput").ap()
    sinN = nc.dram_tensor("sinN", [NK, DH], BF16, kind="ExternalInput").ap()
    scalN_d = nc.dram_tensor("scalN", [128, HG * DH], F32, kind="ExternalInput").ap()
    cscalN_d = nc.dram_tensor("cscalN", [128, HG * DH], F32, kind="ExternalInput").ap()
    outp = nc.dram_tensor("outp", [N, D], F32, kind="ExternalOutput").ap()
    # DRAM scratch for kT/qT between P1 and P2 (rows = h*128 + dh)
    kTs = nc.dram_tensor("kTs", [HG * DH, NK], BF16, kind="Internal").ap()
    qTs = nc.dram_tensor("qTs", [HG * DH, N], BF16, kind="Internal").ap()

    with TileContext(nc) as tc:
      for rep in range(reps):
       with ExitStack() as ctx:
        res = ctx.enter_context(tc.tile_pool(name=f"res{rep}", bufs=1))

        vsb = [res.tile([128, HG * DH], BF16, tag=f"v{i}", name=f"v{i}") for i in range(KB)]
        cos_all = res.tile([128, KB, DH], BF16, tag="cos_all", name="cos_all")
        sin_all = res.tile([128, KB, DH], BF16, tag="sin_all", name="sin_all")
        scalN = res.tile([128, HG * DH], F32, tag="scalN", name="scalN")
        cscalN = res.tile([128, HG * DH], F32, tag="cscalN", name="cscalN")
        ident = res.tile([128, 128], BF16, tag="ident", name="ident")
        ones_fr = res.tile([128, 128], F32R, tag="ones_fr", name="ones_fr")
        ones_f32 = res.tile([128, 128], F32, tag="ones_f32", name="ones_f32")

        def qk_group(work, tpsum, ppsum, scal_tile, pos_chunk, dst, dst_col, dma_eng=None):
            """QK-norm + scale + rope + transpose for one projection group.

            ppsum: PSUM (128 tokens, HG*DH) raw q or k for 4 heads.
            DMAs (dh, token) bf16 into dst[h*128:(h+1)*128, dst_col:+128].
            """
            ssq = work.tile([128, HG], F32, tag="ssq", name="ssq")
            for i in range(HG):
                sq = work.tile([128, DH], F32, tag="sq", name="sq")
                nc.scalar.activation(
                    out=sq, in_=ppsum[:, i * DH:(i + 1) * DH],
                    func=AF.Square, accum_out=ssq[:, i:i + 1],
                )
            nrm = work.tile([128, HG], F32, tag="nrm", name="nrm")
            nc.scalar.activation(out=nrm, in_=ssq, func=AF.Sqrt)
            rn = work.tile([128, HG], F32, tag="rn", name="rn")
            nc.vector.reciprocal(out=rn, in_=nrm)
            stg = work.tile([128, HG, 128], BF16, tag="stg", name="stg")
            for i in range(HG):
                qn = work.tile([128, DH], F32, tag="qn", name="qn")
                # (raw / ||raw||) * scaler, straight out of PSUM in one op
                nc.vector.scalar_tensor_tensor(
                    out=qn, in0=ppsum[:, i * DH:(i + 1) * DH],
                    scalar=rn[:, i:i + 1], in1=scal_tile[:, i * DH:(i + 1) * DH],
                    op0=AluOpType.mult, op1=AluOpType.mult,
                )
                am = work.tile([128, DH], F32, tag="am", name="am")
                bm = work.tile([128, DH], F32, tag="bm", name="bm")
                nc.vector.tensor_mul(am, qn, cos_all[:, pos_chunk, :])
                nc.vector.tensor_mul(bm, qn, sin_all[:, pos_chunk, :])
                rp = work.tile([128, DH], BF16, tag="rp", name="rp")
                nc.vector.tensor_sub(rp[:, 0:64], am[:, 0:64], bm[:, 64:128])
                nc.vector.tensor_add(rp[:, 64:128], bm[:, 0:64], am[:, 64:128])
                tp = tpsum.tile([128, 128], BF16, tag="tp", name="tp")
                nc.tensor.transpose(tp, rp, ident)
                nc.scalar.copy(out=stg[:, i, :], in_=tp)
            (dma_eng or nc.sync).dma_start(
                out=dst[0:HG * DH, dst_col:dst_col + 128].rearrange(
                    "(h p) j -> p h j", p=128),
                in_=stg)

        wctx = ctx.enter_context(ExitStack())
        wres = wctx.enter_context(tc.tile_pool(name=f"wres{rep}", bufs=1))
        wqk = wres.tile([128, NCHUNK, 2 * HG * DH], BF16, tag="wqk", name="wqk")
        wv = wres.tile([128, NCHUNK, HG * DH], F32R, tag="wv", name="wv")

        # ---- P1: self q/k/v (weights fully resident) ----
        # qk_group post-processing for group N is emitted after group N+1's
        # matmul burst, so the PE stream never stalls on the DVE rope chain.
        with tc.tile_pool(name="xp", bufs=6) as xp, \
             tc.tile_pool(name="p1work", bufs=6) as p1work, \
             tc.tile_pool(name="p1psum", bufs=5, space="PSUM") as p1psum, \
             tc.tile_pool(name="p1tp", bufs=3, space="PSUM") as p1tp:
            make_identity(nc, ident)
            nc.vector.memset(ones_f32, 1.0)
            nc.vector.tensor_copy(out=ones_fr, in_=ones_f32)
            pending = []

            def flush_pending():
                while pending:
                    qk_group(p1work, p1tp, *pending.pop(0))

            for st in range(ST):
                xs = []
                xsb = []
                for ss4 in range(4):
                    c0 = st * 512 + ss4 * 128
                    tb = xp.tile([128, NCHUNK, 128], BF16, tag="xb", name="xb")
                    nc.sync.dma_start(
                        out=tb, in_=xTb[:, c0:c0 + 128].rearrange("(c p) j -> p c j", p=128))
                    xsb.append(tb)
                    if st == 0 and ss4 == 0:
                        # weights queue behind the first token subtile; q/k
                        # columns first since the v projection runs last per
                        # subtile -- the first psums need 4.2MB less data
                        nc.sync.dma_start(
                            out=wqk, in_=wqkT.rearrange("(c p) j -> p c j", p=128))
                        nc.sync.dma_start(
                            out=wv, in_=wvT.rearrange("(c p) j -> p c j", p=128))
                        nc.sync.dma_start(out=cos_all, in_=cosN.rearrange("(c p) j -> p c j", p=128))
                        nc.sync.dma_start(out=sin_all, in_=sinN.rearrange("(c p) j -> p c j", p=128))
                        nc.sync.dma_start(out=scalN, in_=scalN_d)
                        nc.sync.dma_start(out=cscalN, in_=cscalN_d)
                for ss4 in range(4):
                    c0 = st * 512 + ss4 * 128
                    t = xp.tile([128, NCHUNK, 128], F32R, tag="xc", name="xc")
                    nc.sync.dma_start(
                        out=t, in_=xT[:, c0:c0 + 128].rearrange("(c p) j -> p c j", p=128))
                    xs.append(t)
                for ss in range(4):
                    tok = st * 4 + ss
                    for grp in range(3):
                        col0 = grp * HG * DH
                        ps = p1psum.tile([128, HG * DH], F32, tag="pp", name="pp")
                        for c in range(NCHUNK):
                            nc.tensor.matmul(
                                ps,
                                lhsT=(xs[ss][:, c, :] if grp == 2 else xsb[ss][:, c, :]),
                                rhs=(wv[:, c, :] if grp == 2
                                     else wqk[:, c, col0:col0 + HG * DH]),
                                start=(c == 0), stop=(c == NCHUNK - 1),
                            )
                        flush_pending()
                        if grp == 0:
                            pending.append((ps, scalN, tok, qTs, tok * 128))
                        elif grp == 1:
                            pending.append((ps, scalN, tok, kTs, tok * 128))
                        else:
                            nc.scalar.copy(out=vsb[tok], in_=ps)
            flush_pending()

        # ---- P0: cross k/v (runs in the P1->P2 transition window) ----
        with tc.tile_pool(name="cres", bufs=1) as cres, \
             tc.tile_pool(name="p0work", bufs=4) as p0work, \
             tc.tile_pool(name="p0psum", bufs=2, space="PSUM") as p0psum, \
             tc.tile_pool(name="p0tp", bufs=2, space="PSUM") as p0tp:
            cc = cres.tile([128, NCHUNK, NCR], BF16, tag="cc", name="cc")
            nc.sync.dma_start(out=cc, in_=cT.rearrange("(c p) j -> p c j", p=128))
            wcK = cres.tile([128, NCHUNK, HG * DH], BF16, tag="wcK", name="wcK")
            wcV = cres.tile([128, NCHUNK, HG * DH], BF16, tag="wcV", name="wcV")
            nc.sync.dma_start(out=wcK, in_=wckvT[:, 0:HG * DH].rearrange("(c p) j -> p c j", p=128))
            nc.sync.dma_start(out=wcV, in_=wckvT[:, HG * DH:].rearrange("(c p) j -> p c j", p=128))
            ps_k = p0psum.tile([128, HG * DH], F32, tag="pk", name="pk")
            ps_v = p0psum.tile([128, HG * DH], F32, tag="pv", name="pv")
            for c in range(NCHUNK):
                nc.tensor.matmul(ps_k, lhsT=cc[:, c, :], rhs=wcK[:, c, :],
                                 start=(c == 0), stop=(c == NCHUNK - 1))
            for c in range(NCHUNK):
                nc.tensor.matmul(ps_v, lhsT=cc[:, c, :], rhs=wcV[:, c, :],
                                 start=(c == 0), stop=(c == NCHUNK - 1))
            nc.scalar.copy(out=vsb[KB - 1], in_=ps_v)
            qk_group(p0work, p0tp, ps_k, cscalN, KB - 1, kTs, N)

        wctx.close()

        # ---- P2: attention + output projection ----
        with tc.tile_pool(name="kqres", bufs=1) as kqres, \
             tc.tile_pool(name="wout", bufs=1) as wores, \
             tc.tile_pool(name="ptp", bufs=30) as ptp, \
             tc.tile_pool(name="otp", bufs=10) as otp, \
             tc.tile_pool(name="accp", bufs=12) as accp, \
             tc.tile_pool(name="p2work", bufs=3) as p2w, \
             tc.tile_pool(name="spsum", bufs=3, space="PSUM") as spsum, \
             tc.tile_pool(name="otsum", bufs=2, space="PSUM") as otsum, \
             tc.tile_pool(name="dnsum", bufs=1, space="PSUM") as dnsum, \
             tc.tile_pool(name="fpsum", bufs=2, space="PSUM") as fpsum:
            kT = [kqres.tile([128, NK], BF16, tag=f"kT{h}", name=f"kT{h}") for h in range(HG)]
            qT = [kqres.tile([128, N], BF16, tag=f"qT{h}", name=f"qT{h}") for h in range(HG)]
            wo = [wores.tile([128, D], F32R, tag=f"wo{h}", name=f"wo{h}") for h in range(HG)]
            for h in range(HG):
                # self columns depend only on P1; cross columns on the
                # (later) cross phase -- split so kb 0..15 never wait on it
                nc.sync.dma_start(out=kT[h][:, 0:N], in_=kTs[h * DH:(h + 1) * DH, 0:N])
                nc.sync.dma_start(out=qT[h], in_=qTs[h * DH:(h + 1) * DH, :])
            for h in range(HG):
                nc.sync.dma_start(out=kT[h][:, N:NK], in_=kTs[h * DH:(h + 1) * DH, N:NK])
            for h in range(HG):
                nc.sync.dma_start(out=wo[h], in_=woutT[h * 128:(h + 1) * 128, :])
            pend_proj = []

            def flush_proj():
                while pend_proj:
                    q0p, oTp = pend_proj.pop(0)
                    for ns in range(4):
                        outsb = p2w.tile([128, D], F32, tag="outsb", name="outsb")
                        for dt_ in range(4):
                            fp = fpsum.tile([128, 512], F32, tag="fp", name="fp")
                            for h in range(HG):
                                nc.tensor.matmul(
                                    fp, lhsT=oTp[h][:, ns * 128:(ns + 1) * 128],
                                    rhs=wo[h][:, dt_ * 512:(dt_ + 1) * 512],
                                    start=(h == 0), stop=(h == HG - 1),
                                )
                            nc.vector.tensor_copy(out=outsb[:, dt_ * 512:(dt_ + 1) * 512], in_=fp)
                        nc.sync.dma_start(out=outp[q0p + ns * 128:q0p + (ns + 1) * 128, :], in_=outsb)

            for qt in range(ST):
                q0 = qt * 512
                oTs = []
                for h in range(HG):
                    pts = []
                    for kb in range(KB):
                        sp = spsum.tile([128, 512], F32, tag="sp", name="sp")
                        nc.tensor.matmul(
                            sp, lhsT=kT[h][:, kb * 128:(kb + 1) * 128],
                            rhs=qT[h][:, q0:q0 + 512], start=True, stop=True,
                        )
                        pt = ptp.tile([128, 512], BF16, tag="pT", name="pT")
                        nc.scalar.activation(out=pt, in_=sp, func=AF.Exp, scale=EXP_SCALE)
                        pts.append(pt)
                    ot = otsum.tile([128, 512], F32, tag="ot", name="ot")
                    for kb in range(KB):
                        nc.tensor.matmul(ot, lhsT=vsb[kb][:, h * 128:(h + 1) * 128],
                                         rhs=pts[kb], start=(kb == 0), stop=(kb == KB - 1))
                    # softmax denominator: tree-sum the 17 p blocks on DVE
                    # (frees the PE from 16 ones-matmuls), then one
                    # ones-stationary matmul for the partition reduction
                    # (broadcast across all 128 partitions).  Emitted after
                    # the oT matmuls: its exp->tree dependency chain resolves
                    # last, so putting it first would stall the PE stream.
                    lvl = list(pts)
                    while len(lvl) > 1:
                        nxt = []
                        for j in range(0, len(lvl) - 1, 2):
                            outdt = F32R if len(lvl) == 2 else BF16
                            s = accp.tile([128, 512], outdt, tag="acc", name="acc")
                            nc.vector.tensor_add(s, lvl[j], lvl[j + 1])
                            nxt.append(s)
                        if len(lvl) % 2:
                            nxt.append(lvl[-1])
                        lvl = nxt
                    dn = dnsum.tile([128, 512], F32, tag="dn", name="dn")
                    nc.tensor.matmul(dn, lhsT=ones_fr, rhs=lvl[0], start=True, stop=True)
                    rc = p2w.tile([128, 512], F32, tag="rc", name="rc")
                    nc.vector.reciprocal(out=rc, in_=dn)
                    oT = otp.tile([128, 512], F32R, tag="oT", name="oT")
                    nc.vector.tensor_mul(oT, ot, rc)
                    oTs.append(oT)
                    if h == 0:
                        flush_proj()
                pend_proj.append((q0, oTs))
            flush_proj()

    nc.finalize()
    return nc


_CACHE = {}


def get_nc(reps=1):
    key = f"nc{reps}"
    if key not in _CACHE:
        _CACHE[key] = _build(reps)
    return _CACHE[key]


def make_in_maps(x, c, w_qkv, w_cross_qkv, w_out, scale, cross_scale):
    x = np.asarray(x, np.float32)
    c = np.asarray(c, np.float32)
    w_qkv = np.asarray(w_qkv, np.float32)
    w_cross_qkv = np.asarray(w_cross_qkv, np.float32)
    w_out = np.asarray(w_out, np.float32)
    scale = np.asarray(scale, np.float32)
    cross_scale = np.asarray(cross_scale, np.float32)

    inv = 1.0 / (10000.0 ** (np.arange(0, DH, 2, dtype=np.float64) / DH))
    ang = np.arange(NK, dtype=np.float64)[:, None] * inv[None, :]
    cosn = np.cos(ang).astype(np.float32)
    sinn = np.sin(ang).astype(np.float32)
    cosN = np.ascontiguousarray(np.concatenate([cosn, cosn], axis=1)).astype(ml_dtypes.bfloat16)
    sinN = np.ascontiguousarray(np.concatenate([sinn, sinn], axis=1)).astype(ml_dtypes.bfloat16)

    xTs = [np.ascontiguousarray(x[b].T) for b in range(B)]
    xTbs = [t.astype(ml_dtypes.bfloat16) for t in xTs]
    cTs = [np.ascontiguousarray(c[b].T).astype(ml_dtypes.bfloat16) for b in range(B)]

    in_maps = []
    for core in range(8):
        b, g = core // 4, core % 4
        rq = slice(512 * g, 512 * (g + 1))
        rk = slice(D + 512 * g, D + 512 * (g + 1))
        rv = slice(2 * D + 512 * g, 2 * D + 512 * (g + 1))
        wqkT = np.ascontiguousarray(
            np.concatenate([w_qkv[rq], w_qkv[rk]], axis=0).T).astype(ml_dtypes.bfloat16)
        wvT = np.ascontiguousarray(w_qkv[rv].T)
        wckvT = np.ascontiguousarray(
            np.concatenate([w_cross_qkv[rk], w_cross_qkv[rv]], axis=0).T
        ).astype(ml_dtypes.bfloat16)
        woutT = np.ascontiguousarray(w_out[:, 512 * g:512 * (g + 1)].T)
        scal = (scale[4 * g:4 * g + 4].reshape(-1) * math.sqrt(D)).astype(np.float32)
        cscal = (cross_scale[4 * g:4 * g + 4].reshape(-1) * math.sqrt(D)).astype(np.float32)
        scalN = np.ascontiguousarray(np.broadcast_to(scal[None, :], (128, HG * DH)))
        cscalN = np.ascontiguousarray(np.broadcast_to(cscal[None, :], (128, HG * DH)))
        in_maps.append({
            "xT": xTs[b], "xTb": xTbs[b], "cT": cTs[b],
            "wqkT": wqkT, "wvT": wvT, "wckvT": wckvT, "woutT": woutT,
            "cosN": cosN, "sinN": sinN,
            "scalN": scalN, "cscalN": cscalN,
        })
    return in_maps


def gather(results, b_out):
    b_out = np.asarray(b_out, np.float32)
    outs = [np.asarray(r["outp"], np.float32) for r in results]
    full = np.stack([sum(outs[0:4]), sum(outs[4:8])], axis=0)
    return (full + b_out[None, None, :]).astype(np.float32)


def kernel(x, c, w_qkv, w_cross_qkv, w_out, b_out, scale, cross_scale):
    nc = get_nc()
    in_maps = make_in_maps(x, c, w_qkv, w_cross_qkv, w_out, scale, cross_scale)
    res = run_bass_kernel_spmd(nc, in_maps, core_ids=list(range(8)))
    return gather(res.results, b_out)
